# revision 31
# baseline (speedup 1.0000x reference)
"""Depthwise 3x3 conv (SAME, channel multiplier 2) on [16,224,224,96] f32,
data-parallel over batch across 8 TRN2 NeuronCores.

Per-core mapping (2 images/core): the conv along H is expressed as a banded
matmul on TensorE — stationary [116,112] band matrix whose 3 diagonals hold
the kernel column k[:, dw, m]; contract dim = 114 input rows (112 out rows +
halo) + 2 bias rows (all-ones coefficients fold the bias add into the PE).
The 3 W-shifts accumulate into PSUM via the moving operand's column offset
(dw*96 in the flattened (w,c) free dim).  f16 operands stream at 1 col/cycle
(2.4 GHz) — 1008 matmuls x 512 cols ~= 218 us tensor floor.

DMA plumbing: the host prepacks x into [8 tiles, 116, 10944] f16 with the
two bias rows baked in, so each tile load is ONE fully-linear 2.54 MB DMA;
the output is written as packed [8 tiles, 112, 21504] f16 linear blocks and
unpacked + upcast to f32 on the host.  PSUM [112, 512] per (chunk, m) pairs
into [112,1024] tiles; DVE/ACT interleave-copy+cast (out ch = 2c+m ->
stride-2 write) into SBUF groups, DMA'd out as contiguous runs.
"""

import sys

sys.path.insert(0, "/opt/trn_rl_repo")

import numpy as np

B, H, W, C = 16, 224, 224, 96
MULT = 2
NCORES = 8
BPC = B // NCORES  # images per core
M = 112            # output rows per h-tile
KP = 116           # contract partitions: 114 x rows + 2 bias rows
WH = 112           # w-half width
COLS = (WH + 2) * C         # 10944 x-tile cols (1-w halo each side)
CHUNK = 512
NCHUNK = WH * C // CHUNK    # 21
NTILE = BPC * 2 * 2         # (b, ht, wh) tiles per core
OC = NCHUNK * CHUNK * MULT  # 21504 packed out cols per tile
PADC = 96                   # DRAM row pad: keeps partition stride != run so
                            # the DMA descriptor spray uses all 16 engines

_cache = {}
XDT = "f16"


def _build():
    import concourse.bacc as bacc
    import concourse.tile as tile
    from concourse import mybir

    f32 = mybir.dt.float32
    f16 = mybir.dt.float16

    nc = bacc.Bacc("TRN2", target_bir_lowering=False, debug=False)
    x_d = nc.dram_tensor("x", [NTILE, KP, COLS + PADC], f16, kind="ExternalInput")
    bands_d = nc.dram_tensor(
        "bands", [KP, 12 * M + PADC], f16, kind="ExternalInput"
    )
    out_d = nc.dram_tensor("out", [NTILE, M, OC], f16, kind="ExternalOutput")

    with tile.TileContext(nc) as tc:
        SCH = 10      # tile-0 starter covers chunks 0..SCH-1
        SCOL = (SCH - 1) * CHUNK + 608  # max col chunk SCH-1 reads
        SPLIT = SCH * CHUNK - 96        # min col chunk SCH reads
        with (
            tc.tile_pool(name="const", bufs=1) as const,
            tc.tile_pool(name="xp", bufs=4) as xp,
            tc.tile_pool(name="op", bufs=4) as op,
            tc.tile_pool(name="ps", bufs=4, space="PSUM") as ps,
        ):
            # 112-partition DMAs spread descriptors over all 16 SDMA engines
            # (the HWDGE only splits partitions into equal per-engine blocks,
            # so 116 partitions would land on just 4 engines); the 4 tail
            # rows go in a separate small DMA.  All HBM loads drain through
            # one per-engine FIFO queue in trigger order, so the bands go
            # absolutely first — they gate the first LDWEIGHTS.
            band_t = const.tile([KP, 12 * M], f16)
            nc.sync.dma_start(band_t[0:112, :], bands_d[0:112, 0 : 12 * M])
            nc.scalar.dma_start(band_t[112:KP, :], bands_d[112:KP, 0 : 12 * M])

            ev = 0  # eviction round-robin DVE/ACT
            for ti in range(NTILE):
                ht = (ti // 2) % 2
                wh = ti % 2
                # jk tap offset in tile cols: col = flat + 96*(jk-1) for
                # wh=0 (tile holds w 0..113), col = flat + 96*(jk+1) for
                # wh=1 (tile holds w 110..223).  The single out-of-range
                # (chunk, jk) at each image w-edge is clipped to N=416 —
                # the dropped 96 columns are exactly the SAME-pad taps.
                joff = -1 if wh == 0 else 1
                jorder = (1, 2, 0) if wh == 0 else (1, 0, 2)
                first = ti == 0
                last = ti == NTILE - 1
                xt = xp.tile([KP, COLS], f16)
                # halo + bias rows ride gpsimd's SWDGE queue: tiny transfers
                # whose buffer-WAR waits must not block the HWDGE rings.
                # Matmuls gate on the WHOLE xt tile's DMAs, so tile 0 gets a
                # separate small starter tile (chunks 0-5) that lands early
                # and lets the PE spin up while the rest streams in.
                if first:
                    st = const.tile([KP, SCOL], f16, tag="starter")
                    nc.gpsimd.dma_start(st[112:KP, :], x_d[ti][112:KP, 0:SCOL])
                    nc.sync.dma_start(st[0:112, :], x_d[ti][0:112, 0:SCOL])
                    nc.gpsimd.dma_start(
                        xt[112:KP, SPLIT:COLS], x_d[ti][112:KP, SPLIT:COLS]
                    )
                    nc.sync.dma_start(
                        xt[0:112, SPLIT:COLS], x_d[ti][0:112, SPLIT:COLS]
                    )
                else:
                    nc.gpsimd.dma_start(xt[112:KP, :], x_d[ti][112:KP, 0:COLS])
                    nc.sync.dma_start(xt[0:112, :], x_d[ti][0:112, 0:COLS])

                if first:
                    groups = (1, 2, 4, 7, 7)
                elif last:
                    groups = (7, 7, 4, 2, 1)
                else:
                    groups = (11, 10)
                ch = 0
                for gsz in groups:
                    og = op.tile([M, 11 * CHUNK * MULT], f16, tag="og")
                    gbase = ch
                    for q in range(gsz):
                        n0 = ch * CHUNK
                        pt = ps.tile([M, 2 * CHUNK], f32)
                        for m in range(MULT):
                            for idx, jk in enumerate(jorder):
                                bi = ht * 6 + m * 3 + jk
                                c0 = n0 + 96 * (jk + joff)
                                p0, p1 = 0, CHUNK
                                if c0 < 0:
                                    p0, c0 = -c0, 0
                                elif c0 + CHUNK > COLS:
                                    p1 = COLS - c0
                                mv = st if (first and ch < SCH) else xt
                                nc.tensor.matmul(
                                    pt[:, m * CHUNK + p0 : m * CHUNK + p1],
                                    band_t[:, bi * M : (bi + 1) * M],
                                    mv[0:KP, c0 : c0 + (p1 - p0)],
                                    start=(idx == 0),
                                    stop=(idx == 2),
                                )
                        src = pt[:, :].rearrange("p (m n) -> p n m", m=2)
                        dst = og[:, q * 1024 : (q + 1) * 1024].rearrange(
                            "p (n m) -> p n m", m=2
                        )
                        if last and ch == NCHUNK - 1:
                            # final chunk: halve across both engines to cut
                            # the drain tail
                            nc.vector.tensor_copy(
                                dst[:, 0:256, :], src[:, 0:256, :]
                            )
                            nc.scalar.copy(dst[:, 256:512, :], src[:, 256:512, :])
                        elif ev % 2 == 0:
                            nc.vector.tensor_copy(dst, src)
                        else:
                            nc.scalar.copy(dst, src)
                        ev += 1
                        ch += 1
                    cb = gbase * CHUNK * MULT
                    glen = gsz * CHUNK * MULT
                    # output DMA triggers live on the sync ring so their
                    # copy-completion waits never block the copy engines
                    nc.sync.dma_start(
                        out_d[ti][:, cb : cb + glen], og[:, 0:glen]
                    )
    nc.compile()
    return nc


def _host_consts(kern, bias):
    kk = np.asarray(kern, np.float32).reshape(3, 3, MULT)  # [dh, dw, m]
    bands = np.zeros((12, KP, M), np.float32)
    for ht in range(2):
        for m in range(MULT):
            for jk in range(3):
                band = bands[ht * 6 + m * 3 + jk]
                for i in range(3):
                    if ht == 0:
                        # tile row k holds x row h=k; out j needs rows j+i-1
                        ks = np.arange(M) + i - 1
                    else:
                        # tile row k holds x row h=110+k; out h=112+j reads
                        # h_in=111+j+i -> k=1+j+i (h_in=224 dropped: SAME pad)
                        ks = np.arange(M) + i + 1
                    js = np.arange(M)
                    sel = (ks >= 0) & (ks <= 113)
                    band[ks[sel], js[sel]] = kk[i, jk, m]
                if jk == 1:
                    band[114 + m, :] = 1.0
    bands = bands.transpose(1, 0, 2).reshape(KP, 12 * M)
    bands = np.pad(bands, ((0, 0), (0, PADC)))
    brows = np.empty((MULT, COLS), np.float32)
    for m in range(MULT):
        brows[m] = np.tile(np.asarray(bias, np.float32)[m::MULT], WH + 2)
    return bands, brows


def _pack_inputs(x, kern, bias):
    """Full f32 x [16,224,224,96] -> per-core packed f16 tiles + bands."""
    bands, brows = _host_consts(kern, bias)
    bands = bands.astype(np.float16)
    brows = brows.astype(np.float16)
    x = np.asarray(x).astype(np.float16)
    in_maps = []
    for core in range(NCORES):
        xc = x[core * BPC : (core + 1) * BPC]
        xa = np.zeros((NTILE, KP, COLS + PADC), np.float16)
        ti = 0
        for b in range(BPC):
            for ht in range(2):
                hs = 0 if ht == 0 else 110
                for wh in range(2):
                    ws = 0 if wh == 0 else 110
                    xa[ti, 0:114, 0:COLS] = xc[
                        b, hs : hs + 114, ws : ws + 114, :
                    ].reshape(114, COLS)
                    xa[ti, 114:KP, 0:COLS] = brows
                    ti += 1
        in_maps.append({"x": xa, "bands": bands})
    return in_maps


def _unpack_output(res):
    """Per-core packed [NTILE, M, OC] f16 -> full [16,224,224,192] f32."""
    outs = []
    for core in range(NCORES):
        oc = np.asarray(res.results[core]["out"])
        # [b, ht, wh, j, wl, c, m] -> [b, ht*j, wh*wl, c*m]
        oc = oc.reshape(BPC, 2, 2, M, WH, C, MULT)
        oc = oc.transpose(0, 1, 3, 2, 4, 5, 6).reshape(BPC, H, W, C * MULT)
        outs.append(oc.astype(np.float32))
    return np.concatenate(outs, axis=0)


def kernel(**inputs):
    in_maps = _pack_inputs(inputs["x"], inputs["kernel"], inputs["bias"])

    if "nc" not in _cache:
        _cache["nc"] = _build()
    nc = _cache["nc"]

    from concourse.bass_utils import run_bass_kernel_spmd

    res = run_bass_kernel_spmd(nc, in_maps, list(range(NCORES)))
    return _unpack_output(res)


# revision 34
# speedup vs baseline: 1.0231x; 1.0231x over previous
"""Depthwise 3x3 conv (SAME, channel multiplier 2) on [16,224,224,96] f32,
data-parallel over batch across 8 TRN2 NeuronCores.

Per-core mapping (2 images/core): the conv along H is expressed as a banded
matmul on TensorE — stationary [116,112] band matrix whose 3 diagonals hold
the kernel column k[:, dw, m]; contract dim = 114 input rows (112 out rows +
halo) + 2 bias rows (all-ones coefficients fold the bias add into the PE).
The 3 W-shifts accumulate into PSUM via the moving operand's column offset
(dw*96 in the flattened (w,c) free dim).  f16 operands stream at 1 col/cycle
(2.4 GHz) — 1008 matmuls x 512 cols ~= 218 us tensor floor.

DMA plumbing: the host prepacks x into [8 tiles, 116, 10944] f16 with the
two bias rows baked in, so each tile load is ONE fully-linear 2.54 MB DMA;
the output is written as packed [8 tiles, 112, 21504] f16 linear blocks and
unpacked + upcast to f32 on the host.  PSUM [112, 512] per (chunk, m) pairs
into [112,1024] tiles; DVE/ACT interleave-copy+cast (out ch = 2c+m ->
stride-2 write) into SBUF groups, DMA'd out as contiguous runs.
"""

import sys

sys.path.insert(0, "/opt/trn_rl_repo")

import numpy as np

B, H, W, C = 16, 224, 224, 96
MULT = 2
NCORES = 8
BPC = B // NCORES  # images per core
M = 112            # output rows per h-tile
KP = 116           # contract partitions: 114 x rows + 2 bias rows
WH = 112           # w-half width
COLS = (WH + 2) * C         # 10944 x-tile cols (1-w halo each side)
CHUNK = 512
NCHUNK = WH * C // CHUNK    # 21
NTILE = BPC * 2 * 2         # (b, ht, wh) tiles per core
OC = NCHUNK * CHUNK * MULT  # 21504 packed out cols per tile
PADC = 96                   # DRAM row pad: keeps partition stride != run so
                            # the DMA descriptor spray uses all 16 engines

_cache = {}
XDT = "f16"


def _build():
    import concourse.bacc as bacc
    import concourse.tile as tile
    from concourse import mybir

    f32 = mybir.dt.float32
    f16 = mybir.dt.float16

    nc = bacc.Bacc("TRN2", target_bir_lowering=False, debug=False)
    x_d = nc.dram_tensor("x", [NTILE, KP, COLS + PADC], f16, kind="ExternalInput")
    bands_d = nc.dram_tensor(
        "bands", [KP, 12 * M + PADC], f16, kind="ExternalInput"
    )
    out_d = nc.dram_tensor("out", [NTILE, M, OC], f16, kind="ExternalOutput")

    with tile.TileContext(nc) as tc:
        MCH = 3       # tile-0 micro-starter covers chunks 0..MCH-1
        MCOL = (MCH - 1) * CHUNK + 608
        MSPLIT = MCH * CHUNK - 96
        SCH = 10      # tile-0 starter covers chunks MCH..SCH-1
        SCOL = (SCH - 1) * CHUNK + 608  # max col chunk SCH-1 reads
        SPLIT = SCH * CHUNK - 96        # min col chunk SCH reads
        with (
            tc.tile_pool(name="const", bufs=1) as const,
            tc.tile_pool(name="xp", bufs=4) as xp,
            tc.tile_pool(name="op", bufs=4) as op,
            tc.tile_pool(name="ps", bufs=4, space="PSUM") as ps,
        ):
            # 112-partition DMAs spread descriptors over all 16 SDMA engines
            # (the HWDGE only splits partitions into equal per-engine blocks,
            # so 116 partitions would land on just 4 engines); the 4 tail
            # rows go in a separate small DMA.  All HBM loads drain through
            # one per-engine FIFO queue in trigger order, so the bands go
            # absolutely first — they gate the first LDWEIGHTS.
            band_t = const.tile([KP, 12 * M], f16)
            nc.sync.dma_start(band_t[0:112, :], bands_d[0:112, 0 : 12 * M])
            nc.scalar.dma_start(band_t[112:KP, :], bands_d[112:KP, 0 : 12 * M])

            ev = 0  # eviction round-robin DVE/ACT
            for ti in range(NTILE):
                ht = (ti // 2) % 2
                wh = ti % 2
                # jk tap offset in tile cols: col = flat + 96*(jk-1) for
                # wh=0 (tile holds w 0..113), col = flat + 96*(jk+1) for
                # wh=1 (tile holds w 110..223).  The single out-of-range
                # (chunk, jk) at each image w-edge is clipped to N=416 —
                # the dropped 96 columns are exactly the SAME-pad taps.
                joff = -1 if wh == 0 else 1
                jorder = (1, 2, 0) if wh == 0 else (1, 0, 2)
                first = ti == 0
                last = ti == NTILE - 1
                xt = xp.tile([KP, COLS], f16)
                # halo + bias rows ride gpsimd's SWDGE queue: tiny transfers
                # whose buffer-WAR waits must not block the HWDGE rings.
                # Matmuls gate on the WHOLE xt tile's DMAs, so tile 0 gets a
                # separate small starter tile (chunks 0-5) that lands early
                # and lets the PE spin up while the rest streams in.
                if first:
                    # three-level load cascade so the PE starts as soon as
                    # the first ~0.4 MB lands
                    mt = const.tile([KP, MCOL], f16, tag="micro")
                    st = const.tile([KP, SCOL], f16, tag="starter")
                    nc.gpsimd.dma_start(mt[112:KP, :], x_d[ti][112:KP, 0:MCOL])
                    nc.sync.dma_start(mt[0:112, :], x_d[ti][0:112, 0:MCOL])
                    nc.gpsimd.dma_start(
                        st[112:KP, MSPLIT:SCOL], x_d[ti][112:KP, MSPLIT:SCOL]
                    )
                    nc.sync.dma_start(
                        st[0:112, MSPLIT:SCOL], x_d[ti][0:112, MSPLIT:SCOL]
                    )
                    nc.gpsimd.dma_start(
                        xt[112:KP, SPLIT:COLS], x_d[ti][112:KP, SPLIT:COLS]
                    )
                    nc.sync.dma_start(
                        xt[0:112, SPLIT:COLS], x_d[ti][0:112, SPLIT:COLS]
                    )
                else:
                    nc.gpsimd.dma_start(xt[112:KP, :], x_d[ti][112:KP, 0:COLS])
                    nc.sync.dma_start(xt[0:112, :], x_d[ti][0:112, 0:COLS])

                if first:
                    groups = (1, 2, 4, 7, 7)
                elif last:
                    groups = (7, 7, 4, 2, 1)
                else:
                    groups = (11, 10)
                ch = 0
                for gsz in groups:
                    og = op.tile([M, 11 * CHUNK * MULT], f16, tag="og")
                    gbase = ch
                    for q in range(gsz):
                        n0 = ch * CHUNK
                        pt = ps.tile([M, 2 * CHUNK], f32)
                        for m in range(MULT):
                            for idx, jk in enumerate(jorder):
                                bi = ht * 6 + m * 3 + jk
                                c0 = n0 + 96 * (jk + joff)
                                p0, p1 = 0, CHUNK
                                if c0 < 0:
                                    p0, c0 = -c0, 0
                                elif c0 + CHUNK > COLS:
                                    p1 = COLS - c0
                                if first and ch < MCH:
                                    mv = mt
                                elif first and ch < SCH:
                                    mv = st
                                else:
                                    mv = xt
                                nc.tensor.matmul(
                                    pt[:, m * CHUNK + p0 : m * CHUNK + p1],
                                    band_t[:, bi * M : (bi + 1) * M],
                                    mv[0:KP, c0 : c0 + (p1 - p0)],
                                    start=(idx == 0),
                                    stop=(idx == 2),
                                )
                        src = pt[:, :].rearrange("p (m n) -> p n m", m=2)
                        dst = og[:, q * 1024 : (q + 1) * 1024].rearrange(
                            "p (n m) -> p n m", m=2
                        )
                        if last and ch == NCHUNK - 1:
                            # final chunk: halve across both engines to cut
                            # the drain tail
                            nc.vector.tensor_copy(
                                dst[:, 0:256, :], src[:, 0:256, :]
                            )
                            nc.scalar.copy(dst[:, 256:512, :], src[:, 256:512, :])
                        elif ev % 2 == 0:
                            nc.vector.tensor_copy(dst, src)
                        else:
                            nc.scalar.copy(dst, src)
                        ev += 1
                        ch += 1
                    cb = gbase * CHUNK * MULT
                    glen = gsz * CHUNK * MULT
                    # output DMA triggers live on the sync ring so their
                    # copy-completion waits never block the copy engines
                    nc.sync.dma_start(
                        out_d[ti][:, cb : cb + glen], og[:, 0:glen]
                    )
    nc.compile()
    return nc


def _host_consts(kern, bias):
    kk = np.asarray(kern, np.float32).reshape(3, 3, MULT)  # [dh, dw, m]
    bands = np.zeros((12, KP, M), np.float32)
    for ht in range(2):
        for m in range(MULT):
            for jk in range(3):
                band = bands[ht * 6 + m * 3 + jk]
                for i in range(3):
                    if ht == 0:
                        # tile row k holds x row h=k; out j needs rows j+i-1
                        ks = np.arange(M) + i - 1
                    else:
                        # tile row k holds x row h=110+k; out h=112+j reads
                        # h_in=111+j+i -> k=1+j+i (h_in=224 dropped: SAME pad)
                        ks = np.arange(M) + i + 1
                    js = np.arange(M)
                    sel = (ks >= 0) & (ks <= 113)
                    band[ks[sel], js[sel]] = kk[i, jk, m]
                if jk == 1:
                    band[114 + m, :] = 1.0
    bands = bands.transpose(1, 0, 2).reshape(KP, 12 * M)
    bands = np.pad(bands, ((0, 0), (0, PADC)))
    brows = np.empty((MULT, COLS), np.float32)
    for m in range(MULT):
        brows[m] = np.tile(np.asarray(bias, np.float32)[m::MULT], WH + 2)
    return bands, brows


def _pack_inputs(x, kern, bias):
    """Full f32 x [16,224,224,96] -> per-core packed f16 tiles + bands."""
    bands, brows = _host_consts(kern, bias)
    bands = bands.astype(np.float16)
    brows = brows.astype(np.float16)
    x = np.asarray(x).astype(np.float16)
    in_maps = []
    for core in range(NCORES):
        xc = x[core * BPC : (core + 1) * BPC]
        xa = np.zeros((NTILE, KP, COLS + PADC), np.float16)
        ti = 0
        for b in range(BPC):
            for ht in range(2):
                hs = 0 if ht == 0 else 110
                for wh in range(2):
                    ws = 0 if wh == 0 else 110
                    xa[ti, 0:114, 0:COLS] = xc[
                        b, hs : hs + 114, ws : ws + 114, :
                    ].reshape(114, COLS)
                    xa[ti, 114:KP, 0:COLS] = brows
                    ti += 1
        in_maps.append({"x": xa, "bands": bands})
    return in_maps


def _unpack_output(res):
    """Per-core packed [NTILE, M, OC] f16 -> full [16,224,224,192] f32."""
    outs = []
    for core in range(NCORES):
        oc = np.asarray(res.results[core]["out"])
        # [b, ht, wh, j, wl, c, m] -> [b, ht*j, wh*wl, c*m]
        oc = oc.reshape(BPC, 2, 2, M, WH, C, MULT)
        oc = oc.transpose(0, 1, 3, 2, 4, 5, 6).reshape(BPC, H, W, C * MULT)
        outs.append(oc.astype(np.float32))
    return np.concatenate(outs, axis=0)


def kernel(**inputs):
    in_maps = _pack_inputs(inputs["x"], inputs["kernel"], inputs["bias"])

    if "nc" not in _cache:
        _cache["nc"] = _build()
    nc = _cache["nc"]

    from concourse.bass_utils import run_bass_kernel_spmd

    res = run_bass_kernel_spmd(nc, in_maps, list(range(NCORES)))
    return _unpack_output(res)
